# revision 30
# baseline (speedup 1.0000x reference)
"""Trainium2 Bass kernel for a post-norm transformer encoder layer.

Contract: kernel(**inputs) takes the FULL fp32 inputs (as produced by the
problem's setup_inputs) and returns the FULL [2, 2048, 512] fp32 output.

Sharding (8 cores, no collectives): core c owns 512 query tokens of batch
c // 4 (slice (c % 4) * 512). Each core recomputes the K/V projections for
its whole batch (2048 tokens) and runs attention + FFN for its 512 queries.

On-device layout is feature-major [feature, token]; the host pre-transposes
all activations and weights so the device never transposes anything.
"""

import numpy as np
import ml_dtypes

D = 512
S = 2048
B = 2
H = 8
HD = 64
F = 2048
EPS = 1e-5
NCORES = 8
SQ = 512          # queries per core
P = 128           # partitions
KD = D // P       # 4   D-tiles
KT = S // P       # 16  key tiles
TB = S // 512     # 4   512-token blocks
FT = F // P       # 16  FFN hidden tiles

BF16 = ml_dtypes.bfloat16

_CACHE = {}
LAST_RESULT = None


def _build_nc():
    import concourse.bacc as bacc
    import concourse.tile as tile
    from concourse import mybir

    bf = mybir.dt.bfloat16
    f32 = mybir.dt.float32
    ACT = mybir.ActivationFunctionType

    nc = bacc.Bacc("TRN2", target_bir_lowering=False, debug=False)

    def din(name, shape, dt=bf):
        return nc.dram_tensor(name, shape, dt, kind="ExternalInput").ap()

    t_qbf = din("qt_bf", [P, KD, SQ])
    t_qf32 = din("qt_f32", [P, KD, SQ], f32)
    t_kt = din("kt", [P, KD, S])
    t_vt = din("vt", [P, KD, S])
    t_wq = din("wqt", [P, KD, D])
    t_wk = din("wkt", [P, KD, D])
    t_wv = din("wvt", [P, KD, D])
    t_wo = din("wot", [HD, H, KD, P])
    t_w1 = din("w1t", [P, KD, F])
    t_w2 = din("w2t", [P, FT, D])
    t_bq = din("bq", [P, KD], f32)
    t_bk = din("bk", [P, KD], f32)
    t_bv = din("bv_bc", [P, D], f32)
    t_bo = din("bo", [P, KD], f32)
    t_b1 = din("b1", [P, FT], f32)
    t_b2 = din("b2", [P, KD], f32)
    t_g1 = din("g1", [P, KD], f32)
    t_be1 = din("be1", [P, KD], f32)
    t_g2 = din("g2", [P, KD], f32)
    t_be2 = din("be2", [P, KD], f32)
    t_out = nc.dram_tensor("outT", [P, KD, SQ], f32, kind="ExternalOutput").ap()

    with tile.TileContext(nc) as tc, \
         tc.tile_pool(name="statics", bufs=1) as SP:
        def st(shape, dt, name):
            return SP.tile(shape, dt, tag=name, name=name)

        # ---- static SBUF tensors (DMAs emitted in first-use order) ----
        ones_bf = st([P, 1], bf, "ones_bf")
        nc.gpsimd.memset(ones_bf, 1.0 / D)
        eps_t = st([1, 1], f32, "eps_t")
        nc.gpsimd.memset(eps_t, EPS)
        warm_sq = st([1, 1], f32, "warm_sq")
        sink0 = st([1, 1], f32, "sink0")
        sink1 = st([1, 1], f32, "sink1")
        warm_rhs = st([1, SQ], bf, "warm_rhs")
        nc.gpsimd.memset(warm_rhs, 0.0)
        ones_row = st([P, HD], f32, "ones_row")
        nc.gpsimd.memset(ones_row, 1.0)
        eps2 = st([1, 1], f32, "eps2")

        bq = st([P, KD], f32, "bq")
        nc.sync.dma_start(out=bq, in_=t_bq)
        qbf = st([P, KD, SQ], bf, "qbf")
        nc.sync.dma_start(out=qbf, in_=t_qbf)
        wq = st([P, KD, D], bf, "wq")
        nc.sync.dma_start(out=wq, in_=t_wq)
        wk = st([P, KD, D], bf, "wk")
        nc.sync.dma_start(out=wk, in_=t_wk)
        kt_sb = st([P, KD, S], bf, "kt_sb")
        nc.sync.dma_start(out=kt_sb[:, :, 0:S // 2], in_=t_kt[:, :, 0:S // 2])
        bk = st([P, KD], f32, "bk")
        nc.sync.dma_start(out=bk, in_=t_bk)
        nc.sync.dma_start(out=kt_sb[:, :, S // 2:S], in_=t_kt[:, :, S // 2:S])
        wv = st([P, KD, D], bf, "wv")
        nc.sync.dma_start(out=wv, in_=t_wv)
        vt_sb = st([P, KD, S], bf, "vt_sb")
        nc.sync.dma_start(out=vt_sb[:, :, 0:S // 2], in_=t_vt[:, :, 0:S // 2])
        bv = st([P, D], f32, "bv")
        nc.sync.dma_start(out=bv, in_=t_bv)
        nc.sync.dma_start(out=vt_sb[:, :, S // 2:S], in_=t_vt[:, :, S // 2:S])
        bo = st([P, KD], f32, "bo")
        nc.sync.dma_start(out=bo, in_=t_bo)
        b1 = st([P, FT], f32, "b1")
        nc.sync.dma_start(out=b1, in_=t_b1)
        b2 = st([P, KD], f32, "b2")
        nc.sync.dma_start(out=b2, in_=t_b2)
        g1 = st([P, KD], f32, "g1")
        nc.sync.dma_start(out=g1, in_=t_g1)
        be1 = st([P, KD], f32, "be1")
        nc.sync.dma_start(out=be1, in_=t_be1)
        g2 = st([P, KD], f32, "g2")
        nc.sync.dma_start(out=g2, in_=t_g2)
        be2 = st([P, KD], f32, "be2")
        nc.sync.dma_start(out=be2, in_=t_be2)
        # activations kept for the whole kernel
        qh = st([P, KD, SQ], bf, "qh")
        kh = st([P, KD, S], bf, "kh")
        vh = st([P, KT, H, HD + 1], bf, "vh")
        avt = st([P, H, SQ], bf, "avt")
        xres = st([P, KD, SQ], f32, "xres")     # later reused as r2
        x1f = st([P, KD, SQ], f32, "x1f")
        x1b = st([P, KD, SQ], bf, "x1b")
        hsb = st([P, FT, SQ], bf, "hsb")
        qf32 = st([P, KD, SQ], f32, "qf32")     # later reused as LN2 out

        # late-phase statics: queued behind phase-1 inputs on purpose
        nc.sync.dma_start(out=qf32, in_=t_qf32)
        wo = st([P, H, KD, P], bf, "wo")
        nc.sync.dma_start(out=wo[0:HD], in_=t_wo)
        w1 = st([P, KD, F], bf, "w1")
        nc.sync.dma_start(out=w1, in_=t_w1)
        w2 = st([P, FT, D], bf, "w2")
        nc.sync.dma_start(out=w2, in_=t_w2)
        SP.seal()

        nc.gpsimd.memset(vh[:, :, :, HD:HD + 1], 1.0)

        # ---------------- phase 1: Q/K/V projections ----------------
        with tc.tile_pool(name="pj", bufs=3, space="PSUM") as pj:
            # late-phase statics: queued behind phase-1 inputs on purpose
            qf32_l = nc.sync.dma_start(out=qf32, in_=t_qf32)
            wo = st([P, H, KD, P], bf, "wo")
            nc.sync.dma_start(out=wo[0:HD], in_=t_wo)
            w1 = st([P, KD, F], bf, "w1")
            nc.sync.dma_start(out=w1, in_=t_w1)
            w2 = st([P, FT, D], bf, "w2")
            nc.sync.dma_start(out=w2, in_=t_w2)
            SP.seal()

            # Q projection: qh[dout, tok] = Wq @ qT
            for dt in range(KD):
                ps = pj.tile([P, SQ], f32, tag="pj")
                for k in range(KD):
                    nc.tensor.matmul(ps, wq[:, k, dt * P:(dt + 1) * P],
                                     qbf[:, k, :], start=(k == 0), stop=(k == KD - 1))
                nc.vector.tensor_scalar_add(qh[:, dt, :], ps, bq[:, dt:dt + 1])

            # K projection over the full batch
            for tb in range(TB):
                tbs = slice(tb * 512, (tb + 1) * 512)
                for dt in range(KD):
                    ps = pj.tile([P, 512], f32, tag="pj")
                    for k in range(KD):
                        nc.tensor.matmul(ps, wk[:, k, dt * P:(dt + 1) * P],
                                         kt_sb[:, k, tbs],
                                         start=(k == 0), stop=(k == KD - 1))
                    nc.vector.tensor_scalar_add(kh[:, dt, tbs], ps, bk[:, dt:dt + 1])

            # V projection, token-major: vh[tok, dv] = vT.T @ WvT
            bv8 = bv.rearrange("p (h d) -> p h d", h=H)
            for tt in range(KT):
                ps = pj.tile([P, D], f32, tag="pj")
                for k in range(KD):
                    nc.tensor.matmul(ps, vt_sb[:, k, tt * P:(tt + 1) * P], wv[:, k, :],
                                     start=(k == 0), stop=(k == KD - 1))
                nc.vector.tensor_add(vh[:, tt, :, 0:HD],
                                     ps.rearrange("p (h d) -> p h d", h=H), bv8)

        # ---------------- phase 2: attention ----------------
        # scoresT[k, q] per head; exp via ACT (scale=1/8); AV accumulates
        # [vh | ones] giving av rows 0:HD and the softmax denominator at HD.
        pop_ctx = tc.tile_pool(name="po", bufs=2, space="PSUM")
        pop = pop_ctx.__enter__()
        with tc.tile_pool(name="att_sb", bufs=1) as asb, \
             tc.tile_pool(name="pj", bufs=2, space="PSUM") as pj, \
             tc.tile_pool(name="sc", bufs=1, space="PSUM") as scp, \
             tc.tile_pool(name="av", bufs=1, space="PSUM") as avp:

            def qk_groups(dt):
                """Projection matmul groups for head pair dt, each ~0.85us."""
                def qgroup():
                    ps = pj.tile([P, SQ], f32, tag="pj", name=f"psq{dt}")
                    for k in range(KD):
                        nc.tensor.matmul(ps, wq[:, k, dt * P:(dt + 1) * P],
                                         qbf[:, k, :], start=(k == 0),
                                         stop=(k == KD - 1))
                    nc.vector.tensor_scalar_add(qh[:, dt, :], ps, bq[:, dt:dt + 1])

                def kgroup(tb):
                    def go():
                        tbs = slice(tb * 512, (tb + 1) * 512)
                        ps = pj.tile([P, 512], f32, tag="pj", name=f"psk{dt}_{tb}")
                        for k in range(KD):
                            nc.tensor.matmul(ps, wk[:, k, dt * P:(dt + 1) * P],
                                             kt_sb[:, k, tbs],
                                             start=(k == 0), stop=(k == KD - 1))
                        nc.vector.tensor_scalar_add(kh[:, dt, tbs], ps,
                                                    bk[:, dt:dt + 1])
                    return go

                return [qgroup] + [kgroup(tb) for tb in range(TB)]

            def v_proj(tt):
                ps = pj.tile([P, D], f32, tag="pj", name=f"psv{tt}")
                for k in range(KD):
                    nc.tensor.matmul(ps, vt_sb[:, k, tt * P:(tt + 1) * P], wv[:, k, :],
                                     start=(k == 0), stop=(k == KD - 1))
                nc.vector.tensor_add(vh[:, tt, :, 0:HD],
                                     ps.rearrange("p (h d) -> p h d", h=H), bv8)

            # Wo runs as interleaved filler MMs inside pair 3 (heads 0-5 are
            # ready by then); po psum tiles live in the pj pool.
            po_tiles = {}

            def wo_mm(dt, h):
                def go():
                    if dt not in po_tiles:
                        po_tiles[dt] = pj.tile([P, SQ], f32, tag="pj",
                                               name=f"po{dt}")
                    nc.tensor.matmul(po_tiles[dt], wo[0:HD, h, dt, :],
                                     avt[0:HD, h, :], start=(h == 0),
                                     stop=(h == H - 1))
                return go

            # keep-warm matmuls: fill the initial DMA wait so the PE ramp is
            # warm when the real work lands. Anchored via the eps chain below.
            warm_ps = scp.tile([P, 2, SQ], f32, tag="sc0", bufs=1, name="warm_ps")
            for w in range(14):
                nc.tensor.matmul(warm_ps[0:1, 0, :], ones_bf[0:1, 0:1],
                                 warm_rhs, start=(w == 0), stop=(w == 13))
            nc.vector.tensor_scalar(out=sink0, in0=warm_ps[0:1, 0, 0:1],
                                    scalar1=0.0, scalar2=0.0,
                                    op0=mybir.AluOpType.mult,
                                    op1=mybir.AluOpType.add)

            # head: projections for pairs 0 and 1 (overlap the input DMAs)
            g0 = qk_groups(0)
            g0[0]()
            warm_ps2 = scp.tile([P, 2, SQ], f32, tag="sc1", bufs=1, name="warm_ps2")
            for w in range(10):
                nc.tensor.matmul(warm_ps2[0:1, 0, :], ones_bf[0:1, 0:1],
                                 warm_rhs, start=(w == 0), stop=(w == 9))
            nc.vector.tensor_scalar(out=sink1, in0=warm_ps2[0:1, 0, 0:1],
                                    scalar1=0.0, scalar2=0.0,
                                    op0=mybir.AluOpType.mult,
                                    op1=mybir.AluOpType.add)
            for g in g0[1:]:
                g()
            for g in qk_groups(1):
                g()
            fillers = []

            for hp in range(KD):  # head pairs (2*hp, 2*hp+1)
                if hp == 1:
                    fillers += qk_groups(2)
                elif hp == 2:
                    fillers += qk_groups(3)
                elif hp == 3:
                    fillers += [wo_mm(dt, h) for dt in (0, 1) for h in range(6)]
                pav0 = avp.tile([P, SQ], f32, tag="av0")
                pav1 = avp.tile([P, SQ], f32, tag="av1")
                prev = None
                for k2 in range(KT // 2):  # pairs of key tiles
                    psc0 = scp.tile([P, 2, SQ], f32, tag="sc0", bufs=1)
                    psc1 = scp.tile([P, 2, SQ], f32, tag="sc1", bufs=1)
                    for i in range(2):
                        kt = 2 * k2 + i
                        ktl = slice(kt * P, (kt + 1) * P)
                        nc.tensor.matmul(psc0[:, i, :], kh[0:HD, hp, ktl],
                                         qh[0:HD, hp, :], start=True, stop=True)
                        nc.tensor.matmul(psc1[:, i, :], kh[HD:P, hp, ktl],
                                         qh[HD:P, hp, :], start=True, stop=True)
                    p0 = asb.tile([P, 2, SQ], bf, tag="p0", bufs=2)
                    nc.scalar.activation(out=p0, in_=psc0, func=ACT.Exp, scale=0.125)
                    p1 = asb.tile([P, 2, SQ], bf, tag="p1", bufs=2)
                    nc.scalar.activation(out=p1, in_=psc1, func=ACT.Exp, scale=0.125)
                    last_p1 = p1
                    if hp == 0:
                        v_proj(2 * k2)
                        v_proj(2 * k2 + 1)
                    elif fillers:
                        fillers.pop(0)()
                    if prev is not None:
                        q0, q1, pk2 = prev
                        for i in range(2):
                            kt = 2 * pk2 + i
                            nc.tensor.matmul(pav0[0:HD + 1, :], vh[:, kt, 2 * hp, :],
                                             q0[:, i, :], start=(kt == 0), stop=False)
                            nc.tensor.matmul(pav1[0:HD + 1, :],
                                             vh[:, kt, 2 * hp + 1, :],
                                             q1[:, i, :], start=(kt == 0), stop=False)
                    prev = (p0, p1, k2)
                q0, q1, pk2 = prev
                for i in range(2):
                    kt = 2 * pk2 + i
                    nc.tensor.matmul(pav0[0:HD + 1, :], vh[:, kt, 2 * hp, :],
                                     q0[:, i, :], start=False, stop=(kt == KT - 1))
                    nc.tensor.matmul(pav1[0:HD + 1, :], vh[:, kt, 2 * hp + 1, :],
                                     q1[:, i, :], start=False, stop=(kt == KT - 1))
                while hp == 3 and fillers:
                    fillers.pop(0)()
                for side, pav in ((0, pav0), (1, pav1)):
                    # partition_broadcast reads only partition 0 correctly on
                    # HW, and the denominator lives at partition HD — use a
                    # K=1 PE matmul to broadcast it instead (reuses a score
                    # psum slot).
                    h = 2 * hp + side
                    rec = asb.tile([P, SQ], f32, tag="rec", bufs=2)
                    nc.vector.reciprocal(rec[HD:HD + 1, :], pav[HD:HD + 1, :])
                    pbc = scp.tile([HD, SQ], f32, tag=f"sc{side}", bufs=1,
                                   name=f"pbc{h}")
                    nc.tensor.matmul(pbc, ones_row[HD:HD + 1, :],
                                     rec[HD:HD + 1, :], start=True, stop=True)
                    nc.vector.tensor_copy(rec[0:HD, :], pbc)
                    nc.vector.tensor_mul(avt[0:HD, h, :], pav[0:HD, :], rec[0:HD, :])

            # Preload the sqrt table set in the idle ACT window between the
            # last exp and LN1; eps2 = warm * 0 + eps keeps the dependencies.
            nc.scalar.activation(out=warm_sq, in_=last_p1[0:1, 1, 0:1], func=ACT.Sqrt)
            nc.vector.tensor_add(warm_sq, warm_sq, sink0)
            nc.vector.tensor_add(warm_sq, warm_sq, sink1)
            nc.vector.tensor_scalar(out=eps2, in0=warm_sq, scalar1=0.0, scalar2=EPS,
                                    op0=mybir.AluOpType.mult, op1=mybir.AluOpType.add)

            # ------------ phase 3: finish Wo + residual ------------
            for dt in range(KD):
                for h in range(H):
                    if not (dt in (0, 1) and h < 6):
                        wo_mm(dt, h)()
                po = po_tiles[dt]
                nc.vector.tensor_scalar_add(po, po, bo[:, dt:dt + 1])
                nc.vector.tensor_add(xres[:, dt, :], po, qf32[:, dt, :])

        def layer_norm(src, gain, beta, dst_f32, dst_bf, stp, tmp):
            """dst = LN(src) * gain + beta over the partition (D) axis."""
            ps1 = stp.tile([1, SQ], f32, tag="s1")
            ps2 = stp.tile([1, SQ], f32, tag="s2")
            for dt in range(KD):
                xb = tmp.tile([P, SQ], bf, tag="xb", bufs=2)
                nc.vector.tensor_copy(xb, src[:, dt, :])
                sq = tmp.tile([P, SQ], bf, tag="sq", bufs=2)
                nc.vector.tensor_mul(sq, xb, xb)
                nc.tensor.matmul(ps1, ones_bf, xb, start=(dt == 0), stop=(dt == KD - 1))
                nc.tensor.matmul(ps2, ones_bf, sq, start=(dt == 0), stop=(dt == KD - 1))
            # ones_bf is 1/D, so ps1 = mean, ps2 = E[x^2] (x scaled by D elsewhere? no: rows)
            mean_sb = tmp.tile([1, SQ], f32, tag="ln_mean")
            nc.vector.tensor_copy(mean_sb, ps1)
            var = tmp.tile([1, SQ], f32, tag="ln_var")
            nc.vector.tensor_mul(var, mean_sb, mean_sb)
            nc.vector.tensor_sub(var, ps2, var)
            sd = tmp.tile([1, SQ], f32, tag="ln_sd")
            nc.scalar.activation(out=sd, in_=var, func=ACT.Sqrt, bias=eps2)
            rstd = tmp.tile([1, SQ], f32, tag="ln_rstd")
            nc.vector.reciprocal(rstd, sd)
            cvec = tmp.tile([1, SQ], f32, tag="ln_c")
            nc.vector.tensor_mul(cvec, mean_sb, rstd)
            pA = tmp.tile([P, SQ], f32, tag="bA")
            nc.gpsimd.partition_broadcast(pA, rstd)
            pC = tmp.tile([P, SQ], f32, tag="bC")
            nc.gpsimd.partition_broadcast(pC, cvec)
            for dt in range(KD):
                t1 = tmp.tile([P, SQ], f32, tag="t1", bufs=2)
                nc.vector.tensor_mul(t1, src[:, dt, :], pA)
                nc.vector.tensor_sub(t1, t1, pC)
                nc.scalar.activation(out=dst_f32[:, dt, :], in_=t1, func=ACT.Identity,
                                     bias=beta[:, dt:dt + 1], scale=gain[:, dt:dt + 1])
                if dst_bf is not None:
                    nc.vector.tensor_copy(dst_bf[:, dt, :], dst_f32[:, dt, :])

        with tc.tile_pool(name="ln1_sb", bufs=1) as tmp1, \
             tc.tile_pool(name="st1", bufs=1, space="PSUM") as stp1:
            for dt in range(KD):
                po = pop.tile([P, SQ], f32, tag="po")
                for h in range(H):
                    nc.tensor.matmul(po, wo[0:HD, h, dt, :], avt[0:HD, h, :],
                                     start=(h == 0), stop=(h == H - 1))
                nc.vector.tensor_scalar_add(po, po, bo[:, dt:dt + 1])
                nc.vector.tensor_add(xres[:, dt, :], po, qf32[:, dt, :])
            layer_norm(xres, g1, be1, x1f, x1b, stp1, tmp1)
        pop_ctx.__exit__(None, None, None)

        # ---------------- phase 4: FFN ----------------
        with tc.tile_pool(name="pf", bufs=5, space="PSUM") as pfp:
            for ft in range(FT):
                pf = pfp.tile([P, SQ], f32, tag="pf")
                for k in range(KD):
                    nc.tensor.matmul(pf, w1[:, k, ft * P:(ft + 1) * P], x1b[:, k, :],
                                     start=(k == 0), stop=(k == KD - 1))
                nc.scalar.activation(out=hsb[:, ft, :], in_=pf, func=ACT.Relu,
                                     bias=b1[:, ft:ft + 1])

        r2 = xres      # dead after LN1 -> reuse for x1 + ffn
        outsb = qf32   # dead after the Wo residual add -> reuse for LN2 out
        with tc.tile_pool(name="ln2_sb", bufs=1) as tmp2, \
             tc.tile_pool(name="py", bufs=3, space="PSUM") as pyp, \
             tc.tile_pool(name="st2", bufs=1, space="PSUM") as stp2:
            for dt in range(KD):
                py = pyp.tile([P, SQ], f32, tag="py")
                for ft in range(FT):
                    nc.tensor.matmul(py, w2[:, ft, dt * P:(dt + 1) * P], hsb[:, ft, :],
                                     start=(ft == 0), stop=(ft == FT - 1))
                nc.vector.tensor_scalar_add(py, py, b2[:, dt:dt + 1])
                nc.vector.tensor_add(r2[:, dt, :], py, x1f[:, dt, :])
            layer_norm(r2, g2, be2, outsb, None, stp2, tmp2)
            for dt in range(KD):
                nc.sync.dma_start(out=t_out[:, dt, :], in_=outsb[:, dt, :])

    nc.compile()
    return nc


def _get_nc():
    if "nc" not in _CACHE:
        _CACHE["nc"] = _build_nc()
    return _CACHE["nc"]


def make_in_maps(q, k, v, Wq, bq, Wk, bk, Wv, bv, Wo, bo, W1, b1, W2, b2,
                 g1, be1, g2, be2):
    f32 = np.float32

    def tile_pd(x, n):  # [n*P] -> [P, n]
        return np.ascontiguousarray(np.asarray(x, f32).reshape(n, P).T)

    def wt(w, cols):  # [in, out] -> [P, in//P, out]
        return np.ascontiguousarray(
            np.asarray(w, f32).T.reshape(-1, P, cols).transpose(1, 0, 2)).astype(BF16)

    shared = {
        "wqt": wt(Wq, D), "wkt": wt(Wk, D), "wvt": wt(Wv, D),
        "w1t": wt(W1, F), "w2t": wt(W2, D),
        "wot": np.ascontiguousarray(
            np.asarray(Wo, f32).T.reshape(H, HD, KD, P).transpose(1, 0, 2, 3)
        ).astype(BF16),
        "bq": tile_pd(bq, KD), "bk": tile_pd(bk, KD),
        "bv_bc": np.ascontiguousarray(
            np.broadcast_to(np.asarray(bv, f32), (P, D))),
        "bo": tile_pd(bo, KD), "b1": tile_pd(b1, FT), "b2": tile_pd(b2, KD),
        "g1": tile_pd(g1, KD), "be1": tile_pd(be1, KD),
        "g2": tile_pd(g2, KD), "be2": tile_pd(be2, KD),
    }

    q = np.asarray(q, f32)
    k = np.asarray(k, f32)
    v = np.asarray(v, f32)
    def fm(x):  # [S, D] -> [P, KD, S] feature-major partition-contiguous
        return np.ascontiguousarray(
            x.T.reshape(KD, P, S).transpose(1, 0, 2)).astype(BF16)

    kts = [fm(k[b]) for b in range(B)]
    vts = [fm(v[b]) for b in range(B)]

    in_maps = []
    for c in range(NCORES):
        b, s0 = c // 4, (c % 4) * SQ
        qt = np.ascontiguousarray(q[b, s0:s0 + SQ, :].T)          # [D, SQ]
        qt4 = np.ascontiguousarray(qt.reshape(KD, P, SQ).transpose(1, 0, 2))
        in_maps.append({
            "qt_bf": qt4.astype(BF16), "qt_f32": qt4,
            "kt": kts[b], "vt": vts[b], **shared,
        })
    return in_maps


def assemble_out(results):
    out = np.empty((B, S, D), np.float32)
    for c in range(NCORES):
        b, s0 = c // 4, (c % 4) * SQ
        # outT [P, KD, SQ]: feature dt*P+p, token t -> out[t, feature]
        out[b, s0:s0 + SQ, :] = results[c]["outT"].transpose(2, 1, 0).reshape(SQ, D)
    return out


def kernel(**inputs):
    global LAST_RESULT
    import os

    from concourse.bass_utils import run_bass_kernel_spmd

    nc = _get_nc()
    in_maps = make_in_maps(**inputs)
    try:
        res = run_bass_kernel_spmd(nc, in_maps, core_ids=list(range(NCORES)))
    except ModuleNotFoundError:
        # BASS_TRACE set but this container has no axon NTFF profile hook
        # (antenv.axon_hooks missing) — rerun untraced.
        os.environ["BASS_NEVER_TRACE"] = "1"
        res = run_bass_kernel_spmd(nc, in_maps, core_ids=list(range(NCORES)))
    LAST_RESULT = res
    return assemble_out(res.results)
